# revision 14
# baseline (speedup 1.0000x reference)
"""Single-head attention (B=8, S=2048, IN=1024, QD=128, VD=1024) on 8 TRN2
NeuronCores, data-parallel over batch (one batch element per core).

Math per core (batch b):
    q = x Wq + bq ; k = x Wk + bk ; v = x Wv + bv
    out = tanh(softmax(q k^T) v)

Layout strategy (all matmuls contract over the partition dim):
  - host pre-transposes x[b] -> xT [IN, S] so projections need no on-chip
    transpose. qT [QD, S] = Wq^T xT, kT likewise, v [S, VD] = xT^T Wv.
  - scores are built TRANSPOSED: sT [t, s] = kT^T qT, so exp(sT) ("E^T")
    is directly the stationary operand of the AV matmul:
        o [s, VD] = (E^T)^T v   (accumulated over 16 t-tiles in PSUM)
    and softmax needs no max-subtraction (|scores| <= ~21, exp is finite
    in fp32) and no transposes.
  - row-denominators: VEC pre-sums the 16 E^T tiles of a block into
    Dp [128, 512] (bf16); one tiny N=1 matmul per 128-row output tile
    (Dp-slice^T @ ones) then gives d = sum over all 2048 t (16 instead of
    256 PE instructions); normalization folds into the final tanh
    activation as a per-partition scale.

Dtypes: q/k/v/scores matmuls run in float32r (fp32 layout, ~11-bit mantissa
rounding on HW, 1 cycle/row vs fp32's 4); E and the AV matmul run in bf16.

Pipelining: (1) warm-up filler matmuls on memset SBUF data (dedicated PSUM
bank) run during every input-DMA wait in the head so the PE's HAM clock
gate reaches and KEEPS 8/8 (2.4 GHz) - input DMAs are issued fine-grained
(xt0/xt1 in 512-col chunks, wq/wk in kt-halves, wv interleaved with xt)
and the projections are split into 4-bank q/k subpasses so filler bursts
fit between them; (2) phases B1/B2 run as two kt-half passes so the
v-projection's first half overlaps the xt4-7/wv DMA stream; (3) phase C
interleaves block n's scores+exp into block n-1's first AV accumulation
loop, block 0's scores ride in phase B2's tail, and each block's
denominator matmuls+reciprocal are emitted inside its first AV t-loop
(so nothing heads the PE queue waiting on the VEC Dp chain); (4) the
last drains are vc-serialized and split in half so the final output DMA
doesn't serialize behind one long transfer.
"""

import numpy as np

import concourse.bacc as bacc
import concourse.mybir as mybir
import concourse.tile as tile
from concourse.bass_utils import run_bass_kernel_spmd

B, S, IN, QD, VD = 8, 2048, 1024, 128, 1024
N_CORES = 8
P = 128
KT = IN // P          # 8 contraction tiles for projections
TT = S // P           # 16 t-tiles
S_BLK = 512           # s-block width for scores/E^T staging
N_BLK = S // S_BLK    # 4 blocks
SS = S_BLK // P       # 4 s-subtiles per block

F32 = mybir.dt.float32
F32R = mybir.dt.float32r
BF16 = mybir.dt.bfloat16

# filler burst sizes (N=128 warm-up matmuls; see docstring)
FILL0 = 56            # initial burst: preamble end -> first xt0 chunks
FILL_KT = 8           # between q-subpass kt groups (xt arrival gaps)
FILL_PRE_V = 24       # before v_pass(0) (wv0-3 arrival gap)

_CACHE: dict = {}


def _build():
    if "nc" in _CACHE:
        return _CACHE["nc"]

    nc = bacc.Bacc("TRN2", target_bir_lowering=False, debug=False,
                   num_devices=N_CORES)

    xT_d = nc.dram_tensor("xT", [IN, S], F32, kind="ExternalInput").ap()
    wq_d = nc.dram_tensor("wq", [P, KT, QD], F32, kind="ExternalInput").ap()
    wk_d = nc.dram_tensor("wk", [P, KT, QD], F32, kind="ExternalInput").ap()
    wv_d = nc.dram_tensor("wv", [P, KT, VD], F32, kind="ExternalInput").ap()
    bq_d = nc.dram_tensor("bq", [QD], F32, kind="ExternalInput").ap()
    bk_d = nc.dram_tensor("bk", [QD], F32, kind="ExternalInput").ap()
    bv_d = nc.dram_tensor("bv", [VD], F32, kind="ExternalInput").ap()
    out_d = nc.dram_tensor("out", [S, VD], F32, kind="ExternalOutput").ap()

    with tile.TileContext(nc) as tc:
        with (
            tc.tile_pool(name="consts", bufs=1) as consts,
            tc.tile_pool(name="xt", bufs=KT) as p_xt,
            tc.tile_pool(name="wv", bufs=KT) as p_wv,
            tc.tile_pool(name="qk", bufs=1) as p_qk,
            tc.tile_pool(name="v", bufs=TT) as p_v,
            tc.tile_pool(name="et", bufs=2 * TT) as p_et,
            tc.tile_pool(name="dp", bufs=2) as p_dp,
            tc.tile_pool(name="o", bufs=2) as p_o,
            tc.tile_pool(name="recip", bufs=4) as p_recip,
            tc.tile_pool(name="ps", bufs=7, space="PSUM") as ps,
            tc.tile_pool(name="fill", bufs=1, space="PSUM") as p_fill,
        ):
            KH = KT // 2
            NSC = S // 512   # 4
            NVC = VD // 512  # 2

            # ---- SBUF destination tiles ----
            wq_sb = consts.tile([P, KT, QD], F32R, tag="wq")
            wk_sb = consts.tile([P, KT, QD], F32R, tag="wk")
            xt_sb = [p_xt.tile([P, S], F32R, tag="xt", name=f"xt{kt}")
                     for kt in range(KT)]
            bq_sb = consts.tile([P, 1], F32, tag="bq")
            bk_sb = consts.tile([P, 1], F32, tag="bk")
            bv_row = consts.tile([1, VD], F32, tag="bv_row")
            bv_sb = consts.tile([P, VD], F32, tag="bv")
            wv_sb = [None] * KT

            # ---- memset constants (no DMA dependency) ----
            ones_sb = consts.tile([P, 1], BF16, tag="ones")
            nc.vector.memset(ones_sb[:], 1.0)
            ones_row = consts.tile([1, P], BF16, tag="ones_row")
            nc.vector.memset(ones_row[:], 1.0)
            fill_w = consts.tile([P, P], BF16, tag="fill_w")
            nc.vector.memset(fill_w[:], 0.0)

            fill_ps = p_fill.tile([P, P], F32, tag="fill", name="fill_ps")

            def fillers(n):
                # PE warm-up / HAM-keepalive matmuls on memset data; results
                # are never read and go to a dedicated PSUM bank.
                for _ in range(n):
                    nc.tensor.matmul(fill_ps[:], fill_w[:], fill_w[:],
                                     start=True, stop=True)

            def load_wv(kt, halves=False):
                t_ = p_wv.tile([P, VD], F32R, tag="wv", name=f"wvt{kt}")
                if halves:
                    # defer the second half-columns: vA's vc0 matmuls only
                    # need cols 0:512 (subtile deps), so its start isn't
                    # gated on the full tile
                    nc.sync.dma_start(out=t_[:, 0:512],
                                      in_=wv_d[:, kt, 0:512].bitcast(F32R))
                else:
                    nc.sync.dma_start(out=t_[:],
                                      in_=wv_d[:, kt, :].bitcast(F32R))
                wv_sb[kt] = t_

            def load_wv_h1(kt):
                nc.sync.dma_start(out=wv_sb[kt][:, 512:VD],
                                  in_=wv_d[:, kt, 512:VD].bitcast(F32R))

            # ---- DMA issue order (sync engine program order) ----
            # first-half columns of xt0-3 + wv0-3 form the minimal working
            # set for proj(sc01)+v(t0-7); late halves and xt4-7 follow.
            def load_xt_cols(kt, c0, c1):
                nc.sync.dma_start(
                    out=xt_sb[kt][:, c0 * 512:c1 * 512],
                    in_=xT_d[kt * P:(kt + 1) * P,
                             c0 * 512:c1 * 512].bitcast(F32R))

            nc.sync.dma_start(out=wq_sb[:, 0:KH, :],
                              in_=wq_d[:, 0:KH, :].bitcast(F32R))
            load_xt_cols(0, 0, 1)
            load_xt_cols(0, 1, 2)
            nc.sync.dma_start(out=wk_sb[:, 0:KH, :],
                              in_=wk_d[:, 0:KH, :].bitcast(F32R))
            load_xt_cols(1, 0, 1)
            load_xt_cols(1, 1, 2)
            # tiny bias loads ride the GpSimd engine's DMA queue so they
            # consume no sync-engine issue slots ahead of the wv stream
            nc.gpsimd.dma_start(out=bq_sb[:],
                                in_=bq_d.rearrange("(p o) -> p o", o=1))
            nc.gpsimd.dma_start(out=bk_sb[:],
                                in_=bk_d.rearrange("(p o) -> p o", o=1))
            nc.gpsimd.dma_start(out=bv_row[:],
                                in_=bv_d.rearrange("(o v) -> o v", o=1))
            load_wv(0)
            load_wv(1)
            load_xt_cols(2, 0, 2)
            load_xt_cols(3, 0, 2)
            load_wv(2, halves=True)
            load_wv(3, halves=True)
            load_wv_h1(2)
            load_wv_h1(3)
            for kt in range(4):
                load_xt_cols(kt, 2, 4)
            for kt in range(4, KT):
                nc.sync.dma_start(out=xt_sb[kt][:],
                                  in_=xT_d[kt * P:(kt + 1) * P, :].bitcast(F32R))
            nc.sync.dma_start(out=wq_sb[:, KH:KT, :],
                              in_=wq_d[:, KH:KT, :].bitcast(F32R))
            nc.sync.dma_start(out=wk_sb[:, KH:KT, :],
                              in_=wk_d[:, KH:KT, :].bitcast(F32R))

            fillers(FILL0)

            def wq_at(kt):
                return wq_sb[:, kt, :]

            def wk_at(kt):
                return wk_sb[:, kt, :]

            # ---- bv broadcast across partitions via a K=1 outer product ----
            def emit_bv_broadcast():
                bv_row_bf = consts.tile([1, VD], BF16, tag="bv_row_bf")
                nc.vector.tensor_copy(bv_row_bf[:], bv_row[:])
                for c in range(NVC):
                    bv_ps = ps.tile([P, 512], F32, tag="ps", name=f"bvps{c}")
                    nc.tensor.matmul(bv_ps[:], ones_row[:],
                                     bv_row_bf[:, c * 512:(c + 1) * 512],
                                     start=True, stop=True)
                    nc.vector.tensor_copy(bv_sb[:, c * 512:(c + 1) * 512],
                                          bv_ps[:])

            # ---- phases B1/B2: projections as two kt-half passes, each
            # split into a q-subpass then k-subpass (4 PSUM banks each) ----
            qT_sb = p_qk.tile([P, S], F32R, tag="qT")
            kT_sb = p_qk.tile([P, S], F32R, tag="kT")

            def proj_subpass(half, which, dst, w_at, bias, scs=None,
                             head_fill=0):
                k0 = half * KH
                scs = list(range(NSC)) if scs is None else list(scs)
                d_ps = {sc: ps.tile([P, 512], F32, tag="ps",
                                    name=f"{which}ps{half}_{sc}")
                        for sc in scs}
                for kt in range(k0, k0 + KH):
                    for sc in scs:
                        nc.tensor.matmul(d_ps[sc][:], w_at(kt),
                                         xt_sb[kt][:, sc * 512:(sc + 1) * 512],
                                         start=(kt == k0),
                                         stop=(kt == k0 + KH - 1))
                    if head_fill and kt < k0 + KH - 1:
                        fillers(head_fill)
                for sc in scs:
                    sl = slice(sc * 512, (sc + 1) * 512)
                    if half == 0:
                        nc.vector.tensor_scalar_add(dst[:, sl], d_ps[sc][:],
                                                    bias)
                    else:
                        nc.vector.tensor_add(dst[:, sl], d_ps[sc][:],
                                             dst[:, sl])

            # ---- phase C helpers (defined early: scores for block 0 are
            # interleaved into phase B2's tail) ----
            dp_state: dict = {}

            def emit_scores_t(sb, t):
                s0 = sb * S_BLK
                st_ps = ps.tile([P, S_BLK], F32, tag="ps", name=f"stps{sb}_{t}")
                nc.tensor.matmul(st_ps[:],
                                 kT_sb[:, t * P:(t + 1) * P],
                                 qT_sb[:, s0:s0 + S_BLK],
                                 start=True, stop=True)
                et = p_et.tile([P, S_BLK], BF16, tag="et", name=f"et{sb}_{t}")
                nc.scalar.activation(out=et[:], in_=st_ps[:],
                                     func=mybir.ActivationFunctionType.Exp)
                # fold into the block's denominator partial-sum on VEC
                st = dp_state.setdefault(sb, [None, 0, None])
                if st[1] == 0:
                    st[2] = et
                elif st[1] == 1:
                    dp = p_dp.tile([P, S_BLK], BF16, tag="dp", name=f"dp{sb}")
                    nc.vector.tensor_add(dp[:], st[2][:], et[:])
                    st[0] = dp
                else:
                    nc.vector.tensor_add(st[0][:], st[0][:], et[:])
                st[1] += 1
                return et

            # ---- phase B2: v [S, VD] = xT^T Wv + bv, stored bf16 ----
            v_sb = [p_v.tile([P, VD], BF16, tag="v", name=f"v{t}")
                    for t in range(TT)]
            et0 = []

            def v_pass(half, interleave0, t0=0, t1=TT):
                k0 = half * KH
                for t in range(t0, t1):
                    vt = v_sb[t]
                    if interleave0 and t >= TT - 11:
                        et0.append(emit_scores_t(0, len(et0)))
                    v_ps = [ps.tile([P, 512], F32, tag="ps",
                                    name=f"vps{half}_{t}_{vc}")
                            for vc in range(NVC)]
                    for kt in range(k0, k0 + KH):
                        xl = xt_sb[kt][:, t * P:(t + 1) * P]
                        for vc in range(NVC):
                            nc.tensor.matmul(
                                v_ps[vc][:], xl,
                                wv_sb[kt][:, vc * 512:(vc + 1) * 512],
                                start=(kt == k0), stop=(kt == k0 + KH - 1))
                    if interleave0 and t >= TT - 5:
                        et0.append(emit_scores_t(0, len(et0)))
                    for vc in range(NVC):
                        sl = slice(vc * 512, (vc + 1) * 512)
                        if half == 0:
                            nc.vector.tensor_add(vt[:, sl], v_ps[vc][:],
                                                 bv_sb[:, sl])
                        else:
                            nc.vector.tensor_add(vt[:, sl], v_ps[vc][:],
                                                 vt[:, sl])

            proj_subpass(0, "q", qT_sb, wq_at, bq_sb[:], scs=(0, 1),
                         head_fill=FILL_KT)
            emit_bv_broadcast()
            proj_subpass(0, "k", kT_sb, wk_at, bk_sb[:], scs=(0, 1))
            fillers(FILL_PRE_V)
            v_pass(0, interleave0=False, t0=0, t1=TT // 2)
            proj_subpass(0, "q", qT_sb, wq_at, bq_sb[:], scs=(2, 3))
            proj_subpass(0, "k", kT_sb, wk_at, bk_sb[:], scs=(2, 3))
            v_pass(0, interleave0=False, t0=TT // 2, t1=TT)
            for kt in range(KT // 2, KT):
                load_wv(kt)
            proj_subpass(1, "q", qT_sb, wq_at, bq_sb[:])
            proj_subpass(1, "k", kT_sb, wk_at, bk_sb[:])
            v_pass(1, interleave0=True)

            # ---- phase C: software-pipelined over s-blocks ----
            def emit_dms(sb):
                dp = dp_state[sb][0]
                d_ps = ps.tile([P, SS], F32, tag="ps", name=f"dps{sb}")
                for ss in range(SS):
                    nc.tensor.matmul(d_ps[:, ss:ss + 1],
                                     dp[:, ss * P:(ss + 1) * P],
                                     ones_sb[:], start=True, stop=True,
                                     skip_group_check=True)
                recip = p_recip.tile([P, SS], F32, tag="recip",
                                     name=f"recip{sb}")
                nc.vector.reciprocal(recip[:], d_ps[:])
                return recip

            def emit_av_ss(sb, ss, et_tiles, recip, interleave_sb=None,
                           interleave_base=0, serialize_vc=False,
                           dms=None, split_last=False):
                n_full = 1 if split_last else NVC
                o_ps = [ps.tile([P, 512], F32, tag="ps", name=f"ops{sb}_{ss}_{i}")
                        for i in range(n_full)]
                nxt = []
                o_sb = p_o.tile([P, VD], F32, tag="o", name=f"osb{sb}_{ss}")
                srow = sb * S_BLK + ss * P

                def drain_vc(vc):
                    lo = vc * 512
                    nc.scalar.activation(
                        out=o_sb[:, lo:lo + 512],
                        in_=o_ps[vc][:],
                        func=mybir.ActivationFunctionType.Tanh,
                        scale=recip[:, ss:ss + 1])
                    nc.sync.dma_start(
                        out=out_d[srow:srow + P, lo:lo + 512],
                        in_=o_sb[:, lo:lo + 512])

                if not serialize_vc:
                    for t in range(TT):
                        if interleave_sb is not None and t % 2 == 0:
                            nxt.append(emit_scores_t(
                                interleave_sb, interleave_base + t // 2))
                        lhs = et_tiles[t][:, ss * P:(ss + 1) * P]
                        for vc in range(NVC):
                            nc.tensor.matmul(o_ps[vc][:], lhs,
                                             v_sb[t][:, vc * 512:(vc + 1) * 512],
                                             start=(t == 0), stop=(t == TT - 1))
                    if dms is not None:
                        recip = dms()
                    for vc in range(NVC):
                        drain_vc(vc)
                else:
                    # tail variant: finish vc0 first so its tanh+DMA overlap
                    # vc1's accumulation
                    for t in range(TT):
                        lhs = et_tiles[t][:, ss * P:(ss + 1) * P]
                        nc.tensor.matmul(o_ps[0][:], lhs, v_sb[t][:, 0:512],
                                         start=(t == 0), stop=(t == TT - 1))
                    drain_vc(0)
                    if not split_last:
                        for t in range(TT):
                            lhs = et_tiles[t][:, ss * P:(ss + 1) * P]
                            nc.tensor.matmul(o_ps[1][:], lhs,
                                             v_sb[t][:, 512:1024],
                                             start=(t == 0), stop=(t == TT - 1))
                        drain_vc(1)
                    else:
                        # final chunk: narrowing accumulation groups in
                        # SEPARATE PSUM tiles (no WAW serialization) so each
                        # piece's tanh+DMA hide under the next piece's matmul
                        # loop; only a 128-col drain is exposed at the end
                        pieces = [(512, 256), (768, 128), (896, 128)]
                        oh = [ps.tile([P, w], F32, tag="ps",
                                      name=f"ohps{sb}_{ss}_{h}")
                              for h, (_, w) in enumerate(pieces)]
                        for h, (lo, w) in enumerate(pieces):
                            for t in range(TT):
                                lhs = et_tiles[t][:, ss * P:(ss + 1) * P]
                                nc.tensor.matmul(
                                    oh[h][:], lhs,
                                    v_sb[t][:, lo:lo + w],
                                    start=(t == 0), stop=(t == TT - 1))
                            nc.scalar.activation(
                                out=o_sb[:, lo:lo + w],
                                in_=oh[h][:],
                                func=mybir.ActivationFunctionType.Tanh,
                                scale=recip[:, ss:ss + 1])
                            nc.sync.dma_start(
                                out=out_d[srow:srow + P, lo:lo + w],
                                in_=o_sb[:, lo:lo + w])
                return nxt, recip

            et_cur = et0
            for sb in range(N_BLK):
                nxt_sb = sb + 1 if sb + 1 < N_BLK else None
                last = sb == N_BLK - 1
                et_nxt, recip = emit_av_ss(sb, 0, et_cur, None,
                                           interleave_sb=nxt_sb,
                                           dms=lambda sb=sb: emit_dms(sb))
                for ssi in range(1, SS):
                    ilv = nxt_sb if ssi == 1 else None
                    r, _ = emit_av_ss(sb, ssi, et_cur, recip,
                                      interleave_sb=ilv,
                                      interleave_base=8,
                                      serialize_vc=(last and ssi >= SS - 2),
                                      split_last=(last and ssi == SS - 1))
                    et_nxt += r
                et_cur = et_nxt

    nc.compile()
    _CACHE["nc"] = nc
    return nc


def _prep_inputs(x, Wq, bq, Wk, bk, Wv, bv):
    x = np.asarray(x, np.float32)
    xT = np.ascontiguousarray(x.transpose(0, 2, 1))          # [B, IN, S]
    wq = np.ascontiguousarray(
        np.asarray(Wq, np.float32).reshape(KT, P, QD).transpose(1, 0, 2))
    wk = np.ascontiguousarray(
        np.asarray(Wk, np.float32).reshape(KT, P, QD).transpose(1, 0, 2))
    wv = np.ascontiguousarray(
        np.asarray(Wv, np.float32).reshape(KT, P, VD).transpose(1, 0, 2))
    shared = {
        "wq": wq, "wk": wk, "wv": wv,
        "bq": np.asarray(bq, np.float32),
        "bk": np.asarray(bk, np.float32),
        "bv": np.asarray(bv, np.float32),
    }
    return [dict(shared, xT=xT[c]) for c in range(N_CORES)]


def run(x, Wq, bq, Wk, bk, Wv, bv, trace=False):
    nc = _build()
    in_maps = _prep_inputs(x, Wq, bq, Wk, bk, Wv, bv)
    res = run_bass_kernel_spmd(nc, in_maps, list(range(N_CORES)), trace=trace)
    out = np.stack([res.results[c]["out"] for c in range(N_CORES)])
    return out.astype(np.float32), res


def kernel(x, Wq, bq, Wk, bk, Wv, bv):
    out, _ = run(x, Wq, bq, Wk, bk, Wv, bv, trace=False)
    return out
